# revision 1
# baseline (speedup 1.0000x reference)
"""Distributed Trainium2 kernel for nn_Curating_of_attention_mask.

Math: batch (3,1280,1280) -> 6400 patches of 16x16 -> per-patch 3x3 channel
gram -> pairwise squared-distance matrix (6400,6400) -> global min/max
normalize -> (1,6400,6400).

Key simplifications:
 - (d - min)/(max - min) is invariant to positive affine rescaling of d, so
   the reference's /768 and /9 factors are dropped.
 - Distances are invariant to subtracting a constant vector from every
   patch's gram, so grams are centered by the expected gram of unit-variance
   noise (256 on the diagonal); this shrinks magnitudes ~30x and with it
   every rounding error.
 - With the 6 unique (centered) gram entries m = [m00,m11,m22,m01,m02,m12]
   and q = m00^2+m11^2+m22^2 + 2*(m01^2+m02^2+m12^2), per patch
     v = [m(6), 1, q],  u = [-2*m_diag(3), -4*m_off(3), q, 1]
   give raw[i,j] = u_i . v_j = q_i + q_j - 2*<gram_i, gram_j>, and
   out = raw*s + t with s = 1/(M-m), t = -m*s from one AllReduce(max) of
   [max, -min].

Sharding: patch dim across 8 cores (core k owns image rows [160k,160k+160)
= patches [800k,800k+800)).  Each core builds u/v for its 800 patches,
AllGathers v (8x800 f32), computes its [800,6400] slice of raw twice
(pass 1 reduces min/max, pass 2 applies the affine and writes out).
"""

import numpy as np

import concourse.bass as bass
import concourse.mybir as mybir
import concourse.tile as tile
from concourse.bass_utils import run_bass_kernel_spmd

F32 = mybir.dt.float32
F32R = mybir.dt.float32r
I32 = mybir.dt.int32
AX = mybir.AluOpType
AFT = mybir.ActivationFunctionType

N_CORES = 8
C, H, W = 3, 1280, 1280
PS = 16
HP, WP = H // PS, W // PS            # 80, 80
N = HP * WP                          # 6400
H_LOC = H // N_CORES                 # 160 image rows per core
TP = H_LOC // PS                     # 10 patch-rows per core
N_LOC = TP * WP                      # 800 patches per core
K = 8                                # feature dim of u/v
PP = PS * PS                         # 256 pixels per patch

# feature order: diagonals first, then off-diagonals
PAIRS = [(0, 0), (1, 1), (2, 2), (0, 1), (0, 2), (1, 2)]
NEG_BIG = -3.0e38
CENTER = 256.0

# "f32r" = float32r matmuls (2 cyc/row, inputs rounded to ~TF32 precision)
# "f32"  = plain float32 matmuls (4 cyc/row, exact)
MM_MODE = "f32r"

# output tiling
M_TILES = [(ms, min(128, N_LOC - ms)) for ms in range(0, N_LOC, 128)]   # 7
N_TILES = [(ns, min(512, N - ns)) for ns in range(0, N, 512)]           # 13
NTOT = len(M_TILES) * len(N_TILES)                                      # 91

# walrus in this container accepts at most 1 sync-wait command per
# instruction; Tile's tail drain can carry several.  Split extras onto
# preceding NOPs on the same engine (stream order preserves semantics).
_MAX_WAITS = 1


def _split_sync_waits(nc):
    n_fixed = 0
    for func in nc.m.functions:
        for bb in func.blocks:
            new_insts = []
            for inst in bb.instructions:
                si = inst.sync_info
                if si is not None and si.on_wait and len(si.on_wait) > _MAX_WAITS:
                    waits = list(si.on_wait)
                    keep = waits[-_MAX_WAITS:]
                    extra = waits[:-_MAX_WAITS]
                    chunks = [
                        extra[i : i + _MAX_WAITS]
                        for i in range(0, len(extra), _MAX_WAITS)
                    ]
                    for ci, chunk in enumerate(chunks):
                        nop = mybir.InstNoOp(
                            name=f"{inst.name}-waitsplit-{ci}",
                            engine=inst.engine,
                            ins=[],
                            outs=[],
                            sync_info=mybir.SyncInfo(on_wait=chunk, on_update=[]),
                        )
                        new_insts.append(nop)
                        n_fixed += 1
                    si.on_wait = keep
                new_insts.append(inst)
            bb.instructions[:] = new_insts
    return n_fixed


def _build():
    nc = bass.Bass(num_devices=N_CORES)
    x = nc.dram_tensor("x", [C, H_LOC, W], F32, kind="ExternalInput")
    out = nc.dram_tensor("out", [N_LOC, N], F32, kind="ExternalOutput")
    groups = [list(range(N_CORES))]

    with tile.TileContext(nc, num_cores=N_CORES) as tc:
        with (
            tc.tile_pool(name="dram", bufs=1, space="DRAM") as dpool,
            tc.tile_pool(name="cst", bufs=1) as cst,
            tc.tile_pool(name="scr", bufs=2) as scrp,
            tc.tile_pool(name="obig", bufs=2) as obig,
            tc.tile_pool(name="ps", bufs=5, space="PSUM") as psp,
        ):
            v_dram = dpool.tile([K, N_LOC], F32, name="v_dram")
            vall = dpool.tile([K * N_CORES, N_LOC], F32, addr_space="Shared",
                              name="vall")
            cc_in = dpool.tile([1, 8], F32, name="cc_in")
            cc_out = dpool.tile([1, 8], F32, addr_space="Shared", name="cc_out")

            # identity matrix for PE transposes
            iota2d = cst.tile([128, 128], F32, name="iota2d")
            nc.gpsimd.iota(iota2d[:, :], pattern=[[1, 128]], base=0,
                           channel_multiplier=0,
                           allow_small_or_imprecise_dtypes=True)
            iota_col = cst.tile([128, 1], F32, name="iota_col")
            nc.gpsimd.iota(iota_col[:, :], pattern=[[0, 1]], base=0,
                           channel_multiplier=1,
                           allow_small_or_imprecise_dtypes=True)
            ident = cst.tile([128, 128], F32, name="ident")
            nc.vector.tensor_scalar(
                out=ident[:, :], in0=iota2d[:, :], scalar1=iota_col[:, 0:1],
                scalar2=None, op0=AX.is_equal,
            )

            # va/ua hold per-patch features, column layout 8t+slot
            va = cst.tile([WP, K * TP], F32, name="va")
            ua = cst.tile([WP, K * TP], F32, name="ua")
            va_r = va.rearrange("p (t s) -> p t s", s=K)
            ua_r = ua.rearrange("p (t s) -> p t s", s=K)

            # ---- phase A: load patches in 2 half-slabs per channel so gram
            # compute overlaps the loads; pools close afterwards to free SBUF
            HT = TP // 2  # patch-rows per half
            with (
                tc.tile_pool(name="phA", bufs=1) as phap,
                tc.tile_pool(name="prodp", bufs=2) as prodp,
            ):
                xall = []
                dma_engines = [nc.sync, nc.scalar, nc.gpsimd]
                for c in range(C):
                    xc = phap.tile([WP, TP * PP], F32, name=f"xall{c}")
                    xc_r = xc.rearrange("w (t a b) -> w t a b", a=PS, b=PS)
                    for h in range(2):
                        dma_engines[c].dma_start(
                            xc_r[:, HT * h : HT * (h + 1), :, :],
                            x[
                                c, PS * HT * h : PS * HT * (h + 1), :
                            ].rearrange("(t a) (w b) -> w t a b", a=PS, b=PS),
                        )
                    xall.append(xc)

                # gram features: product then grouped per-t reduce, per half
                for h in range(2):
                    tsl = slice(HT * h, HT * (h + 1))
                    csl = slice(HT * PP * h, HT * PP * (h + 1))
                    for f, (a, b) in enumerate(PAIRS):
                        prodb = prodp.tile([WP, HT * PP], F32, name="prodb",
                                           tag="prodb")
                        if a == b:
                            nc.scalar.activation(
                                prodb[:, :], xall[a][:, csl], AFT.Square
                            )
                        else:
                            nc.vector.tensor_mul(
                                prodb[:, :], xall[a][:, csl], xall[b][:, csl]
                            )
                        nc.vector.tensor_reduce(
                            out=va_r[:, tsl, f : f + 1],
                            in_=prodb.rearrange("p (t e) -> p t e", e=PP),
                            axis=mybir.AxisListType.X,
                            op=AX.add,
                        )
            # center diagonal gram entries (features 0..2)
            nc.vector.tensor_scalar_add(va_r[:, :, 0:3], va_r[:, :, 0:3], -CENTER)
            # q = sum(diag^2) + 2*sum(off^2)
            msq = cst.tile([WP, 6 * TP], F32, name="msq")
            msq_r = msq.rearrange("p (t s) -> p t s", s=6)
            nc.vector.tensor_mul(msq_r[:, :, :], va_r[:, :, 0:6], va_r[:, :, 0:6])
            qd = cst.tile([WP, TP], F32, name="qd")
            qo = cst.tile([WP, TP], F32, name="qo")
            nc.vector.tensor_reduce(
                out=qd[:, :], in_=msq_r[:, :, 0:3],
                axis=mybir.AxisListType.X, op=AX.add,
            )
            nc.vector.tensor_reduce(
                out=qo[:, :], in_=msq_r[:, :, 3:6],
                axis=mybir.AxisListType.X, op=AX.add,
            )
            nc.vector.scalar_tensor_tensor(
                out=va_r[:, :, 7:8].rearrange("p t s -> p (t s)"),
                in0=qo[:, :], scalar=2.0, in1=qd[:, :],
                op0=AX.mult, op1=AX.add,
            )
            nc.vector.memset(va_r[:, :, 6:7], 1.0)
            # u features
            nc.scalar.activation(ua_r[:, :, 0:3], va_r[:, :, 0:3], AFT.Copy,
                                 scale=-2.0)
            nc.scalar.activation(ua_r[:, :, 3:6], va_r[:, :, 3:6], AFT.Copy,
                                 scale=-4.0)
            nc.scalar.activation(ua_r[:, :, 6:7], va_r[:, :, 7:8], AFT.Copy)
            nc.vector.memset(ua_r[:, :, 7:8], 1.0)

            # transpose [80, 80] feature blocks -> [8, 800] operand layouts
            v_sbT = cst.tile([K, N_LOC], F32, name="v_sbT")
            lhsT = cst.tile([K, N_LOC], F32, name="lhsT")
            for src_r, dst in ((va_r, v_sbT), (ua_r, lhsT)):
                for t in range(TP):
                    ps_tr = psp.tile([K, WP], F32, name="ps_tr", tag="ps_tr",
                                     bufs=2)
                    nc.tensor.transpose(
                        ps_tr[0:K, 0:WP],
                        src_r[:, t, :],
                        ident[0:WP, 0:WP],
                    )
                    nc.vector.tensor_copy(
                        dst[:, WP * t : WP * (t + 1)], ps_tr[0:K, 0:WP]
                    )
            nc.sync.dma_start(v_dram[:, :], v_sbT[:, :])

            # ---- all-gather v across cores ----
            nc.gpsimd.collective_compute(
                "AllGather",
                AX.bypass,
                replica_groups=groups,
                ins=[v_dram.opt()],
                outs=[vall.opt()],
            )

            rhs = cst.tile([K, N], F32, name="rhs")
            nc.sync.dma_start(
                rhs.rearrange("f (r l) -> f r l", l=N_LOC),
                vall.rearrange("(r f) l -> f r l", f=K),
            )

            if MM_MODE == "f32r":
                mm_rhs = cst.tile([K, N], F32R, name="mm_rhs")
                nc.vector.tensor_copy(mm_rhs[:, :], rhs[:, :])
                mm_lhsT = cst.tile([K, N_LOC], F32R, name="mm_lhsT")
                nc.vector.tensor_copy(mm_lhsT[:, :], lhsT[:, :])
            else:
                mm_rhs, mm_lhsT = rhs, lhsT

            # ---- pass 1: min/max of raw ----
            # one [128, 6400] staging row-block per M-tile; per-block flat
            # max on GpSimd (otherwise idle) and flat negated-min on DVE
            MPAD = 8
            racc = cst.tile([128, 2 * MPAD], F32, name="racc")
            nc.vector.memset(racc[:, :], NEG_BIG)
            for i, (ms, mh) in enumerate(M_TILES):
                scb = scrp.tile([128, N], F32, name="scb", tag="scb")
                for ns, nw in N_TILES:
                    ps_t = psp.tile([128, 512], F32, name="ps1", tag="ps")
                    nc.tensor.matmul(
                        ps_t[0:mh, 0:nw],
                        lhsT=mm_lhsT[:, ms : ms + mh],
                        rhs=mm_rhs[:, ns : ns + nw],
                        start=True,
                        stop=True,
                    )
                    nc.scalar.activation(
                        scb[0:mh, ns : ns + nw], ps_t[0:mh, 0:nw], AFT.Copy
                    )
                nc.vector.tensor_reduce(
                    out=racc[0:mh, i : i + 1],
                    in_=scb[0:mh, :],
                    axis=mybir.AxisListType.X,
                    op=AX.max,
                )
                nc.vector.tensor_reduce(
                    out=racc[0:mh, MPAD + i : MPAD + i + 1],
                    in_=scb[0:mh, :],
                    axis=mybir.AxisListType.X,
                    op=AX.min,
                    negate=True,
                )

            # fold -> [128,2] = [max, -min]; partitions via DRAM transpose; cores
            gpair = cst.tile([128, 2], F32, name="gpair")
            nc.vector.tensor_reduce(
                out=gpair[:, :],
                in_=racc.rearrange("p (h n) -> p h n", h=2),
                axis=mybir.AxisListType.X,
                op=AX.max,
            )
            gp_dram = dpool.tile([128, 2], F32, name="gp_dram")
            nc.sync.dma_start(gp_dram[:, :], gpair[:, :])
            gpT = cst.tile([2, 128], F32, name="gpT")
            nc.sync.dma_start(gpT[:, :], gp_dram.rearrange("w f -> f w"))
            gg = cst.tile([2, 1], F32, name="gg")
            nc.vector.tensor_reduce(
                out=gg[:, :], in_=gpT[:, :], axis=mybir.AxisListType.X, op=AX.max
            )
            sc8 = cst.tile([1, 8], F32, name="sc8")
            nc.vector.memset(sc8[:, :], NEG_BIG)
            nc.sync.dma_start(cc_in[:, :], sc8[:, :])
            nc.sync.dma_start(cc_in[0:1, 0:2].rearrange("a b -> b a"), gg[:, :])
            nc.gpsimd.collective_compute(
                "AllReduce",
                AX.max,
                replica_groups=groups,
                ins=[cc_in.opt()],
                outs=[cc_out.opt()],
            )
            mm_sb = cst.tile([1, 2], F32, name="mm_sb")
            nc.sync.dma_start(mm_sb[:, :], cc_out[0:1, 0:2])
            # broadcast [1,2] -> [128,2] via ones-matmul into PSUM
            ones_col = cst.tile([1, 128], F32, name="ones_col")
            nc.vector.memset(ones_col[:, :], 1.0)
            ps_bc = psp.tile([128, 2], F32, name="ps_bc", tag="ps_bc", bufs=1)
            nc.tensor.matmul(
                ps_bc[:, :],
                lhsT=ones_col[0:1, :],
                rhs=mm_sb[0:1, :],
                start=True,
                stop=True,
            )
            # s = 1/(M - m); t = -m * s   (bc_sb = [M, -m] per partition)
            bc_sb = cst.tile([128, 2], F32, name="bc_sb")
            nc.vector.tensor_copy(bc_sb[:, :], ps_bc[:, :])
            rng = cst.tile([128, 1], F32, name="rng")
            nc.vector.tensor_add(rng[:, :], bc_sb[:, 0:1], bc_sb[:, 1:2])
            s_sb = cst.tile([128, 1], F32, name="s_sb")
            nc.vector.reciprocal(s_sb[:, :], rng[:, :])
            t_sb = cst.tile([128, 1], F32, name="t_sb")
            nc.vector.tensor_mul(t_sb[:, :], bc_sb[:, 1:2], s_sb[:, :])

            # ---- pass 2: recompute, normalize (ACT/DVE alternating), write
            # out in full row blocks
            k2 = 0
            for ms, mh in M_TILES:
                ob = obig.tile([128, N], F32, name="ob", tag="ob")
                for ns, nw in N_TILES:
                    ps_t = psp.tile([128, 512], F32, name="ps2", tag="ps")
                    nc.tensor.matmul(
                        ps_t[0:mh, 0:nw],
                        lhsT=mm_lhsT[:, ms : ms + mh],
                        rhs=mm_rhs[:, ns : ns + nw],
                        start=True,
                        stop=True,
                    )
                    if k2 % 2 == 0:
                        nc.scalar.activation(
                            ob[0:mh, ns : ns + nw],
                            ps_t[0:mh, 0:nw],
                            AFT.Identity,
                            bias=t_sb[0:mh, 0:1],
                            scale=s_sb[0:mh, 0:1],
                        )
                    else:
                        nc.vector.tensor_scalar(
                            out=ob[0:mh, ns : ns + nw],
                            in0=ps_t[0:mh, 0:nw],
                            scalar1=s_sb[0:mh, 0:1],
                            scalar2=t_sb[0:mh, 0:1],
                            op0=AX.mult,
                            op1=AX.add,
                        )
                    k2 += 1
                nc.sync.dma_start(out[ms : ms + mh, :], ob[0:mh, :])

    _split_sync_waits(nc)
    return nc


_NC_CACHE = []


def kernel(batch: np.ndarray) -> np.ndarray:
    batch = np.asarray(batch, dtype=np.float32)
    assert batch.shape == (C, H, W)
    if not _NC_CACHE:
        _NC_CACHE.append(_build())
    nc = _NC_CACHE[0]
    in_maps = [
        {"x": np.ascontiguousarray(batch[:, k * H_LOC : (k + 1) * H_LOC, :])}
        for k in range(N_CORES)
    ]
    res = run_bass_kernel_spmd(nc, in_maps, core_ids=list(range(N_CORES)))
    full = np.concatenate([res.results[k]["out"] for k in range(N_CORES)], axis=0)
    return full[None].astype(np.float32)



# revision 7
# speedup vs baseline: 1.4079x; 1.4079x over previous
"""Distributed Trainium2 kernel for nn_Curating_of_attention_mask.

Math: batch (3,1280,1280) -> 6400 patches of 16x16 -> per-patch 3x3 channel
gram -> pairwise squared-distance matrix (6400,6400) -> global min/max
normalize -> (1,6400,6400).

Simplifications (validated in numpy against the fp32 reference,
absmax-normalized error 5.0e-3 < 2e-2):
 - (d - min)/(max - min) is invariant to positive affine rescaling, so the
   reference's /768 and /9 factors are dropped.
 - Grams are centered by 256 on the diagonal (expected gram of unit
   variance noise), shrinking magnitudes ~30x before any rounding.
 - The true distance matrix has min exactly 0 (diagonal); the fp min only
   differs by rounding noise ~1e-5 relative to the max, so min is taken
   as 0: out = raw / max.  One AllReduce(max) of a scalar suffices.
 - With the 6 unique centered gram entries m and
   q = m0^2+m1^2+m2^2 + 2*(m3^2+m4^2+m5^2), the feature vectors
     v = [m(6), 1, q],  u = [-2*m_diag(3), -4*m_off(3), q, 1]
   give raw[i,j] = u_i . v_j = q_i + q_j - 2*<gram_i, gram_j>_w.

Schedule per core (rows [800k, 800k+800) of the output):
 - Load the 160-row image slab as six contiguous [80,1280] tiles (full DMA
   bandwidth; the old patch-gather DMA ran at 24 GB/s).
 - Per channel-pair products in row layout, grouped free-axis reduce over
   the 16 patch columns (DVE), then a tiny mask matmul on the PE contracts
   the 16 image rows per patch: grams with zero strided-DMA cost.
 - u/v features in bf16; AllGather of v (bf16 halves the payload).
 - Pass 1: u^T v in bf16 (1 cyc/col on the PE) into [128,2048] PSUM tiles;
   PSUM->SBUF bf16 staging split between ACT (plain copies) and DVE
   (tensor_tensor_reduce = copy fused with a running max); running max of
   the other chunks via 2x-mode bf16 tensor_max chains.
 - Cross-partition fold via gpsimd.partition_all_reduce, AllReduce(max).
 - Pass 2: out = staged * (1/M) split ACT/DVE, overlapped with the output
   DMA (the 20.5 MB/core write is the 57 us roofline of this kernel).
"""

import numpy as np

import concourse.bass as bass
import concourse.bass_isa as bass_isa
import concourse.mybir as mybir
import concourse.tile as tile
from concourse.bass_utils import run_bass_kernel_spmd

F32 = mybir.dt.float32
BF16 = mybir.dt.bfloat16
AX = mybir.AluOpType
AFT = mybir.ActivationFunctionType

N_CORES = 8
C, H, W = 3, 1280, 1280
PS = 16
HP, WP = H // PS, W // PS            # 80, 80
N = HP * WP                          # 6400
H_LOC = H // N_CORES                 # 160 image rows per core
TP = H_LOC // PS                     # 10 patch-rows per core
N_LOC = TP * WP                      # 800 patches per core
K = 8                                # feature dim of u/v
RT = 2                               # row-tiles per core
RROWS = H_LOC // RT                  # 80 image rows per row-tile
TT = TP // RT                        # 5 patch-rows per row-tile

PAIRS = [(0, 0), (1, 1), (2, 2), (0, 1), (0, 2), (1, 2)]
NEG_BIG = -3.0e38
CENTER = 256.0

M_TILES = [(ms, min(128, N_LOC - ms)) for ms in range(0, N_LOC, 128)]   # 7
# psum chunk groups per M-tile; each is filled by ceil(w/512) matmuls and
# drained by one wide ACT/DVE instruction
GROUPS = [(0, 2048), (2048, 2048), (4096, 2048), (6144, 256)]
MM = 512                             # matmul moving-dim cap

# walrus in this container accepts at most 1 sync-wait command per
# instruction; Tile's tail drain can carry several.  Split extras onto
# preceding NOPs on the same engine (stream order preserves semantics).
_MAX_WAITS = 1


def _split_sync_waits(nc):
    n_fixed = 0
    for func in nc.m.functions:
        for bb in func.blocks:
            new_insts = []
            for inst in bb.instructions:
                si = inst.sync_info
                if si is not None and si.on_wait and len(si.on_wait) > _MAX_WAITS:
                    waits = list(si.on_wait)
                    keep = waits[-_MAX_WAITS:]
                    extra = waits[:-_MAX_WAITS]
                    chunks = [
                        extra[i : i + _MAX_WAITS]
                        for i in range(0, len(extra), _MAX_WAITS)
                    ]
                    for ci, chunk in enumerate(chunks):
                        nop = mybir.InstNoOp(
                            name=f"{inst.name}-waitsplit-{ci}",
                            engine=inst.engine,
                            ins=[],
                            outs=[],
                            sync_info=mybir.SyncInfo(on_wait=chunk, on_update=[]),
                        )
                        new_insts.append(nop)
                        n_fixed += 1
                    si.on_wait = keep
                new_insts.append(inst)
            bb.instructions[:] = new_insts
    return n_fixed


def _build():
    nc = bass.Bass(num_devices=N_CORES)
    x = nc.dram_tensor("x", [C, H_LOC, W], F32, kind="ExternalInput")
    out = nc.dram_tensor("out", [N_LOC, N], F32, kind="ExternalOutput")
    groups = [list(range(N_CORES))]

    with tile.TileContext(nc, num_cores=N_CORES) as tc:
        with (
            tc.tile_pool(name="dram", bufs=1, space="DRAM") as dpool,
            tc.tile_pool(name="cst", bufs=1) as cst,
        ):
            v_dram = dpool.tile([K, N_LOC], BF16, name="v_dram")
            vall = dpool.tile([K * N_CORES, N_LOC], BF16, addr_space="Shared",
                              name="vall")
            cc_in = dpool.tile([1, 8], F32, name="cc_in")
            cc_out = dpool.tile([1, 8], F32, addr_space="Shared", name="cc_out")

            # identity matrix for PE transposes
            iota2d = cst.tile([128, 128], F32, name="iota2d")
            nc.gpsimd.iota(iota2d[:, :], pattern=[[1, 128]], base=0,
                           channel_multiplier=0,
                           allow_small_or_imprecise_dtypes=True)
            iota_col = cst.tile([128, 1], F32, name="iota_col")
            nc.gpsimd.iota(iota_col[:, :], pattern=[[0, 1]], base=0,
                           channel_multiplier=1,
                           allow_small_or_imprecise_dtypes=True)
            ident = cst.tile([128, 128], F32, name="ident")
            nc.vector.tensor_scalar(
                out=ident[:, :], in0=iota2d[:, :], scalar1=iota_col[:, 0:1],
                scalar2=None, op0=AX.is_equal,
            )

            # amask[r, t] = 1 if image row r belongs to patch-row t (r//16==t)
            adiff = cst.tile([RROWS, TT], F32, name="adiff")
            nc.gpsimd.iota(adiff[:, :], pattern=[[PS, TT]], base=0,
                           channel_multiplier=-1,
                           allow_small_or_imprecise_dtypes=True)
            am1 = cst.tile([RROWS, TT], F32, name="am1")
            nc.vector.tensor_scalar(out=am1[:, :], in0=adiff[:, :],
                                    scalar1=0.0, scalar2=None, op0=AX.is_le)
            am2 = cst.tile([RROWS, TT], F32, name="am2")
            nc.vector.tensor_scalar(out=am2[:, :], in0=adiff[:, :],
                                    scalar1=float(-PS), scalar2=None,
                                    op0=AX.is_gt)
            amask = cst.tile([RROWS, TT], F32, name="amask")
            nc.vector.tensor_mul(amask[:, :], am1[:, :], am2[:, :])

            # per-patch feature tiles, layout [80 w, (rt, tt, slot)]
            vfeat = cst.tile([WP, RT * TT * K], F32, name="vfeat")
            ufeat = cst.tile([WP, RT * TT * K], F32, name="ufeat")
            vfeat_r = vfeat.rearrange("p (r t s) -> p r t s", t=TT, s=K)
            ufeat_r = ufeat.rearrange("p (r t s) -> p r t s", t=TT, s=K)

            # ---- phase A: contiguous slab loads + grams in row layout ----
            with (
                tc.tile_pool(name="phA", bufs=1) as phap,
                tc.tile_pool(name="prodp", bufs=3) as prodp,
                tc.tile_pool(name="gsump", bufs=4) as gsump,
                tc.tile_pool(name="psA", bufs=1, space="PSUM") as psA,
            ):
                dma_engines = [nc.sync, nc.scalar, nc.gpsimd]
                xs = []
                for c in range(C):
                    row = []
                    for rt in range(RT):
                        xt = phap.tile([RROWS, W], F32, name=f"x{c}_{rt}")
                        dma_engines[c].dma_start(
                            xt[:, :], x[c, RROWS * rt : RROWS * (rt + 1), :]
                        )
                        row.append(xt)
                    xs.append(row)

                # grams: per (rt, pair) product -> free-reduce over the 16
                # patch cols -> PE mask-matmul contracts the 16 rows
                gpsum = psA.tile([RROWS, RT * 6 * TT], F32, name="gpsum",
                                 tag="gp")
                gpsum_r = gpsum.rearrange("p (r f t) -> p r f t", f=6, t=TT)
                n_cross = 0
                for rt in range(RT):
                    for f, (a, b) in enumerate(PAIRS):
                        prod = prodp.tile([RROWS, W], F32, name="prod",
                                          tag="prod")
                        if a == b:
                            nc.scalar.activation(
                                prod[:, :], xs[a][rt][:, :], AFT.Square
                            )
                        else:
                            # crosses go to the otherwise-idle GpSimd
                            eng = nc.gpsimd if n_cross % 3 != 2 else nc.vector
                            eng.tensor_mul(
                                prod[:, :], xs[a][rt][:, :], xs[b][rt][:, :]
                            )
                            n_cross += 1
                        gsum = gsump.tile([RROWS, WP], F32, name="gsum",
                                          tag="gsum")
                        nc.vector.tensor_reduce(
                            out=gsum[:, :],
                            in_=prod.rearrange("p (w b) -> p w b", b=PS),
                            axis=mybir.AxisListType.X,
                            op=AX.add,
                        )
                        nc.tensor.matmul(
                            gpsum_r[:, rt, f, :],
                            lhsT=gsum[:, :],
                            rhs=amask[:, :],
                            start=True,
                            stop=True,
                        )

                # grams to SBUF, diagonal entries centered
                gfeat = cst.tile([WP, RT * 6 * TT], F32, name="gfeat")
                nc.scalar.activation(gfeat[:, :], gpsum[:, :], AFT.Copy)
                gfeat_r = gfeat.rearrange("p (r f t) -> p r f t", f=6, t=TT)
                nc.vector.tensor_scalar_add(
                    gfeat_r[:, :, 0:3, :], gfeat_r[:, :, 0:3, :], -CENTER
                )

                # v features: [m(6), 1, q]
                nc.vector.tensor_copy(
                    vfeat_r[:, :, :, 0:6],
                    gfeat.rearrange("p (r f t) -> p r t f", f=6, t=TT),
                )
                msq = cst.tile([WP, RT * TT * 6], F32, name="msq")
                msq_r = msq.rearrange("p (r t f) -> p r t f", t=TT, f=6)
                nc.vector.tensor_mul(
                    msq_r[:, :, :, :], vfeat_r[:, :, :, 0:6],
                    vfeat_r[:, :, :, 0:6]
                )
                qd = cst.tile([WP, RT * TT], F32, name="qd")
                qo = cst.tile([WP, RT * TT], F32, name="qo")
                qd_r = qd.rearrange("p (r t) -> p r t", t=TT)
                qo_r = qo.rearrange("p (r t) -> p r t", t=TT)
                nc.vector.tensor_reduce(
                    out=qd_r[:, :, :], in_=msq_r[:, :, :, 0:3],
                    axis=mybir.AxisListType.X, op=AX.add,
                )
                nc.vector.tensor_reduce(
                    out=qo_r[:, :, :], in_=msq_r[:, :, :, 3:6],
                    axis=mybir.AxisListType.X, op=AX.add,
                )
                nc.vector.scalar_tensor_tensor(
                    out=vfeat_r[:, :, :, 7:8].rearrange("p r t s -> p (r t s)"),
                    in0=qo[:, :], scalar=2.0, in1=qd[:, :],
                    op0=AX.mult, op1=AX.add,
                )
                nc.vector.memset(vfeat_r[:, :, :, 6:7], 1.0)

                # u features: [-2 m_diag, -4 m_off, q, 1]
                nc.scalar.activation(ufeat_r[:, :, :, 0:3],
                                     vfeat_r[:, :, :, 0:3], AFT.Copy,
                                     scale=-2.0)
                nc.scalar.activation(ufeat_r[:, :, :, 3:6],
                                     vfeat_r[:, :, :, 3:6], AFT.Copy,
                                     scale=-4.0)
                nc.scalar.activation(ufeat_r[:, :, :, 6:7],
                                     vfeat_r[:, :, :, 7:8], AFT.Copy)
                nc.vector.memset(ufeat_r[:, :, :, 7:8], 1.0)

                # transpose [80, 8] feature blocks -> [8, 800] bf16 operands;
                # v first so the AllGather can start as early as possible
                v_bf = cst.tile([K, N_LOC], BF16, name="v_bf")
                u_bf = cst.tile([K, N_LOC], BF16, name="u_bf")
                for src_r, dst in ((vfeat_r, v_bf), (ufeat_r, u_bf)):
                    for t10 in range(TP):
                        rt, tt = divmod(t10, TT)
                        ps_tr = psA.tile([K, WP], F32, name="ps_tr",
                                         tag="ps_tr", bufs=2)
                        nc.tensor.transpose(
                            ps_tr[0:K, 0:WP],
                            src_r[:, rt, tt, :],
                            ident[0:WP, 0:WP],
                        )
                        eng = nc.vector if t10 % 2 == 0 else nc.scalar
                        if eng is nc.scalar:
                            nc.scalar.activation(
                                dst[:, WP * t10 : WP * (t10 + 1)],
                                ps_tr[0:K, 0:WP], AFT.Copy,
                            )
                        else:
                            nc.vector.tensor_copy(
                                dst[:, WP * t10 : WP * (t10 + 1)],
                                ps_tr[0:K, 0:WP],
                            )
                    if dst is v_bf:
                        nc.sync.dma_start(v_dram[:, :], v_bf[:, :])
                        # ---- all-gather v across cores (bf16) ----
                        nc.gpsimd.collective_compute(
                            "AllGather",
                            AX.bypass,
                            replica_groups=groups,
                            ins=[v_dram.opt()],
                            outs=[vall.opt()],
                        )

            # constants used by pass 1 (built during the AllGather)
            acc_bf = cst.tile([128, 2048], BF16, name="acc_bf")
            nc.vector.memset(acc_bf[:, :], NEG_BIG)
            sc8 = cst.tile([1, 8], F32, name="sc8")
            nc.vector.memset(sc8[:, :], NEG_BIG)
            nc.sync.dma_start(cc_in[:, :], sc8[:, :])

            # de-interleave gathered v into [8, 6400]
            rhs_bf = cst.tile([K, N], BF16, name="rhs_bf")
            nc.sync.dma_start(
                rhs_bf.rearrange("f (r l) -> f r l", l=N_LOC),
                vall.rearrange("(r f) l -> f r l", f=K),
            )

            with (
                tc.tile_pool(name="stg", bufs=1) as stg,
                tc.tile_pool(name="obig", bufs=2) as obig,
                tc.tile_pool(name="psmm", bufs=1, space="PSUM") as psmm,
            ):
                staged = [
                    stg.tile([128, N], BF16, name=f"stg{m}")
                    for m in range(len(M_TILES))
                ]
                # ---- pass 1: raw = u^T v, staged bf16 + running max ----
                # copies are split ACT/DVE so both engines finish together;
                # the running max reads the bf16 staging at the DVE 2x rate
                for m, (ms, mh) in enumerate(M_TILES):
                    for gi, (c0, cw) in enumerate(GROUPS):
                        ps_t = psmm.tile([128, 2048], F32, name="ps_mm",
                                         tag="mm", bufs=2)
                        for s0 in range(0, cw, MM):
                            sw = min(MM, cw - s0)
                            nc.tensor.matmul(
                                ps_t[0:mh, s0 : s0 + sw],
                                lhsT=u_bf[:, ms : ms + mh],
                                rhs=rhs_bf[:, c0 + s0 : c0 + s0 + sw],
                                start=True,
                                stop=True,
                            )
                        if gi == 2:
                            # split this chunk's copy between ACT and DVE
                            nc.scalar.activation(
                                staged[m][0:mh, c0 : c0 + 1152],
                                ps_t[0:mh, 0:1152], AFT.Copy,
                            )
                            nc.vector.tensor_copy(
                                staged[m][0:mh, c0 + 1152 : c0 + cw],
                                ps_t[0:mh, 1152:cw],
                            )
                        elif gi == 3:
                            nc.vector.tensor_copy(
                                staged[m][0:mh, c0 : c0 + cw],
                                ps_t[0:mh, 0:cw],
                            )
                        else:
                            nc.scalar.activation(
                                staged[m][0:mh, c0 : c0 + cw],
                                ps_t[0:mh, 0:cw], AFT.Copy,
                            )
                        nc.vector.tensor_max(
                            acc_bf[0:mh, 0:cw],
                            acc_bf[0:mh, 0:cw],
                            staged[m][0:mh, c0 : c0 + cw],
                        )

                # fold: acc_bf -> [128,1], partitions -> 1
                gmax = cst.tile([128, 1], F32, name="gmax")
                nc.vector.tensor_reduce(
                    out=gmax[:, :], in_=acc_bf[:, :],
                    axis=mybir.AxisListType.X, op=AX.max,
                )
                # cross-partition max via a DRAM transpose round-trip
                gp_dram = dpool.tile([128, 1], F32, name="gp_dram")
                nc.sync.dma_start(gp_dram[:, :], gmax[:, :])
                gpT = cst.tile([1, 128], F32, name="gpT")
                nc.sync.dma_start(gpT[:, :], gp_dram.rearrange("p o -> o p"))
                g1 = cst.tile([1, 1], F32, name="g1")
                nc.vector.tensor_reduce(
                    out=g1[:, :], in_=gpT[:, :],
                    axis=mybir.AxisListType.X, op=AX.max,
                )
                nc.sync.dma_start(cc_in[0:1, 0:1], g1[:, :])
                nc.gpsimd.collective_compute(
                    "AllReduce",
                    AX.max,
                    replica_groups=groups,
                    ins=[cc_in.opt()],
                    outs=[cc_out.opt()],
                )
                msb = cst.tile([1, 1], F32, name="msb")
                nc.sync.dma_start(msb[:, :], cc_out[0:1, 0:1])
                # broadcast M to all partitions via a ones-matmul
                ones_row = cst.tile([1, 128], F32, name="ones_row")
                nc.vector.memset(ones_row[:, :], 1.0)
                ps_bc = psmm.tile([128, 2048], F32, name="ps_bc", tag="mm",
                                  bufs=2)
                nc.tensor.matmul(
                    ps_bc[0:128, 0:1],
                    lhsT=ones_row[0:1, :],
                    rhs=msb[0:1, 0:1],
                    start=True,
                    stop=True,
                )
                mb = cst.tile([128, 1], F32, name="mb")
                nc.vector.tensor_copy(mb[:, :], ps_bc[0:128, 0:1])
                s_sb = cst.tile([128, 1], F32, name="s_sb")
                nc.vector.reciprocal(s_sb[:, :], mb[:, :])

                # ---- pass 2: out = staged / M, overlapped with the DMA ----
                for m, (ms, mh) in enumerate(M_TILES):
                    ob = obig.tile([128, N], F32, name="ob", tag="ob")
                    nc.scalar.activation(
                        ob[0:mh, 0:3200], staged[m][0:mh, 0:3200],
                        AFT.Copy, scale=s_sb[0:mh, 0:1],
                    )
                    nc.vector.tensor_scalar(
                        out=ob[0:mh, 3200:N],
                        in0=staged[m][0:mh, 3200:N],
                        scalar1=s_sb[0:mh, 0:1],
                        scalar2=None,
                        op0=AX.mult,
                    )
                    nc.sync.dma_start(out[ms : ms + mh, :], ob[0:mh, :])

    _split_sync_waits(nc)
    return nc


_NC_CACHE = []


def kernel(batch: np.ndarray) -> np.ndarray:
    batch = np.asarray(batch, dtype=np.float32)
    assert batch.shape == (C, H, W)
    if not _NC_CACHE:
        _NC_CACHE.append(_build())
    nc = _NC_CACHE[0]
    in_maps = [
        {"x": np.ascontiguousarray(batch[:, k * H_LOC : (k + 1) * H_LOC, :])}
        for k in range(N_CORES)
    ]
    res = run_bass_kernel_spmd(nc, in_maps, core_ids=list(range(N_CORES)))
    full = np.concatenate([res.results[k]["out"] for k in range(N_CORES)], axis=0)
    return full[None].astype(np.float32)
